# revision 13
# baseline (speedup 1.0000x reference)
"""MAE self-attention Trainium2 Bass kernel, v2 (compacted keys).

Sharding: 8 cores = batch(2) x head-groups(4 groups of 3 heads).
vs v1: masked keys are compacted away on host (2176 -> ~1792 padded key
slots), removing ~18% of work on every engine; kv projection runs in bf16
(k-chains deduplicated: heads h0+h1 share one 128-partition chain, h2 uses
the duplicated-columns chain); the (key j == q) diagonal zeroing uses
host-built per-tile mask windows (batch-uniform bounds, data-driven values)
multiplied into pt.
Everything else follows v1: scores^T via row-packed bf16 matmuls, ACT exp
with per-key bias (masked/pad -> 0), pv accumulates [v | 1] so row 64 is the
softmax denominator; host divides, transposes, reassembles.
"""

import ml_dtypes
import numpy as np

import concourse.bacc as bacc
import concourse.bass as bass  # noqa: F401
import concourse.mybir as mybir
import concourse.tile as tile
from concourse.bass_utils import run_bass_kernel_spmd

F32 = mybir.dt.float32
F32R = mybir.dt.float32r
BF16 = mybir.dt.bfloat16
FP8 = mybir.dt.float8e4

B = 2
S = 2048          # queries
HID = 768
H = 12
D = 64
G = 3             # heads per core
NCORE = 8
KC = HID // 128   # 6 contraction chunks
NEG = -10000.0
SCALE = 0.125     # D ** -0.5
DW = 256          # diag mask window width

Exp = mybir.ActivationFunctionType.Exp

# W layout: [Wk_h0 Wk_h1 | Wk_h2 Wk_h2 | Wv]  (bf16, 448 cols)
WCOLS = 448


def _build_nc(ntc, los, ablate=(), reps=1):
    """ntc: number of 128-wide key tiles; los: per-tile diag window starts."""
    ab = set(ablate)
    lookahead = 1 if ("la1" in ab or "psv2" in ab) else 2
    psa_bufs = 2 if "psv2" in ab else 3
    psv_bufs = 2 if "psv2" in ab else 1
    ptdt = BF16 if "ptbf" in ab else F32R
    skc = ntc * 128
    nc = bacc.Bacc(None, target_bir_lowering=False)

    xT_d = nc.dram_tensor("xT", [HID, skc], BF16, kind="ExternalInput")
    qT_d = nc.dram_tensor("qT", [G * D, S], BF16, kind="ExternalInput")
    w_d = nc.dram_tensor("W", [HID, WCOLS], BF16, kind="ExternalInput")
    bk_d = nc.dram_tensor("bk", [128, G], F32, kind="ExternalInput")
    bv_d = nc.dram_tensor("bv", [128, G * D], F32, kind="ExternalInput")
    kb_d = nc.dram_tensor("kb", [128, ntc], F32, kind="ExternalInput")
    dm_d = nc.dram_tensor("dm", [128, ntc * DW], BF16, kind="ExternalInput")
    out_d = nc.dram_tensor("outT", [G, D + 1, S], F32, kind="ExternalOutput")

    with tile.TileContext(nc) as tc:
        with (
            tc.tile_pool(name="const", bufs=1) as cpool,
            tc.tile_pool(name="work", bufs=3) as wpool,
            tc.tile_pool(name="ovec", bufs=2) as opool,
            tc.tile_pool(name="psA", bufs=psa_bufs, space="PSUM") as psa,
            tc.tile_pool(name="psV", bufs=psv_bufs, space="PSUM") as psv,
        ):
            xT_sb = cpool.tile([128, KC, skc], BF16)
            w_sb = cpool.tile([128, KC, WCOLS], BF16)
            qT_sb = cpool.tile([128, G, S], BF16)
            bk_sb = cpool.tile([128, G], F32)
            bv_sb = cpool.tile([128, G * D], F32)
            kb_sb = cpool.tile([128, ntc], F32)
            dm_sb = cpool.tile([128, ntc, DW], BF16)
            kT_sb = cpool.tile([128, G, skc], BF16)
            v_sb = cpool.tile([128, ntc, G, D + 1], ptdt)

            # Loads split across the two HWDGE trigger engines (SP / ACT).
            # W leads (first PE work needs it); dm trails (first use ~16us in).
            for kc in range(KC):
                nc.sync.dma_start(
                    out=w_sb[:, kc, :], in_=w_d[kc * 128 : (kc + 1) * 128, :]
                )
            nc.sync.dma_start(out=bk_sb, in_=bk_d[:, :])
            nc.sync.dma_start(out=bv_sb, in_=bv_d[:, :])
            nc.sync.dma_start(out=kb_sb, in_=kb_d[:, :])
            nc.scalar.dma_start(out=qT_sb[0:D, 0, :], in_=qT_d[0:D, :])
            nc.scalar.dma_start(out=qT_sb[D:128, 0, :], in_=qT_d[0:D, :])
            for c0, c1 in ((0, min(1024, skc)), (min(1024, skc), skc)):
                if c0 >= c1:
                    continue
                for kc in range(KC):
                    eng = nc.sync if kc % 2 == 0 else nc.scalar
                    eng.dma_start(
                        out=xT_sb[:, kc, c0:c1],
                        in_=xT_d[kc * 128 : (kc + 1) * 128, c0:c1],
                    )
            nc.sync.dma_start(
                out=dm_sb, in_=dm_d[:, :].rearrange("p (t w) -> p t w", t=ntc)
            )
            for h in range(1, G):
                nc.scalar.dma_start(
                    out=qT_sb[0:D, h, :], in_=qT_d[h * D : (h + 1) * D, :]
                )
                nc.scalar.dma_start(
                    out=qT_sb[D:128, h, :], in_=qT_d[h * D : (h + 1) * D, :]
                )

            for rep in range(reps):
                # ---- kv projection ----
                # Chain A: W cols [h0 | h1] -> psum rows 0-63 = k(h0),
                # 64-127 = k(h1). Chain B: W cols [h2 | h2] (dup).
                # kT halves not covered by aligned DVE writes are filled by
                # SBUF->SBUF DMA duplication.
                kchunks = [(0, 1024)] if skc <= 1024 else [
                    (0, 1024), (1024, skc)]

                def proj_k_chain(chain, c0, c1):
                    ps = psa.tile([128, 1024], F32, tag="ps")
                    wofs = 0 if chain == 0 else 128
                    csz = c1 - c0
                    for kc in range(KC):
                        nn = 0
                        while nn < csz:
                            nsz = min(512, csz - nn)
                            nc.tensor.matmul(
                                ps[:, nn : nn + nsz],
                                w_sb[:, kc, wofs : wofs + 128],
                                xT_sb[:, kc, c0 + nn : c0 + nn + nsz],
                                start=(kc == 0),
                                stop=(kc == KC - 1),
                            )
                            nn += nsz
                    if chain == 0:
                        nc.vector.tensor_scalar_add(
                            kT_sb[0:D, 0, c0:c1], ps[0:D, 0:csz],
                            bk_sb[0:D, 0:1],
                        )
                        nc.vector.tensor_scalar_add(
                            kT_sb[D:128, 1, c0:c1], ps[D:128, 0:csz],
                            bk_sb[D:128, 1:2],
                        )
                        nc.sync.dma_start(
                            out=kT_sb[D:128, 0, c0:c1], in_=kT_sb[0:D, 0, c0:c1]
                        )
                        nc.sync.dma_start(
                            out=kT_sb[0:D, 1, c0:c1], in_=kT_sb[D:128, 1, c0:c1]
                        )
                    else:
                        nc.vector.tensor_scalar_add(
                            kT_sb[:, 2, c0:c1], ps[:, 0:csz], bk_sb[:, 2:3]
                        )

                def proj_v_tile(t):
                    ps = psa.tile([128, 1024], F32, tag="ps")
                    for kc in range(KC):
                        nc.tensor.matmul(
                            ps[:, 0 : G * D],
                            xT_sb[:, kc, t * 128 : (t + 1) * 128],
                            w_sb[:, kc, 256:WCOLS],
                            start=(kc == 0),
                            stop=(kc == KC - 1),
                        )
                    nc.vector.tensor_add(
                        v_sb[:, t, :, 0:D],
                        ps[:, 0 : G * D].rearrange("p (h d) -> p h d", h=G),
                        bv_sb.rearrange("p (h d) -> p h d", h=G),
                    )
                    if ptdt is F32R:
                        nc.vector.memset(
                            v_sb[:, t, :, D : D + 1].bitcast(F32), 1.0
                        )
                    else:
                        nc.vector.memset(v_sb[:, t, :, D : D + 1], 1.0)

                # ---- attention ----
                steps = [(h, half, t) for h in range(G) for half in range(2)
                         for t in range(ntc)]
                n_steps = len(steps)
                pv_tiles = {}

                def emit_scores(i):
                    h, half, t = steps[i]
                    q0 = half * 1024
                    ps = psa.tile([128, 1024], F32, tag="ps")
                    nc.tensor.matmul(
                        ps[:, 0:512],
                        kT_sb[0:D, h, t * 128 : (t + 1) * 128],
                        qT_sb[0:D, h, q0 : q0 + 512],
                        start=True,
                        stop=True,
                        tile_position=(0, 0),
                    )
                    nc.tensor.matmul(
                        ps[:, 512:1024],
                        kT_sb[D:128, h, t * 128 : (t + 1) * 128],
                        qT_sb[D:128, h, q0 + 512 : q0 + 1024],
                        start=True,
                        stop=True,
                        tile_position=(64, 0),
                    )
                    return ps

                def emit_exp(i, ps):
                    h, half, t = steps[i]
                    q0 = half * 1024
                    pt = wpool.tile([128, 1024], ptdt, tag="pt")
                    if "act8" in ab:
                        nc.scalar.activation(
                            pt[:, 0:128], ps[:, 0:128], Exp,
                            bias=kb_sb[:, t : t + 1], scale=SCALE,
                        )
                        return pt
                    nc.scalar.activation(
                        pt, ps, Exp, bias=kb_sb[:, t : t + 1], scale=SCALE
                    )
                    # zero the scattered compacted diagonal via mask window
                    lo = los[t]
                    a = max(lo, q0)
                    b = min(lo + DW, q0 + 1024, S)
                    if a < b and "nodiag" not in ab:
                        nc.vector.tensor_mul(
                            pt[:, a - q0 : b - q0],
                            pt[:, a - q0 : b - q0],
                            dm_sb[:, t, a - lo : b - lo],
                        )
                    return pt

                def emit_pv(i, pt):
                    h, half, t = steps[i]
                    q0 = half * 1024
                    if t == 0:
                        pv_tiles[(h, half)] = psv.tile(
                            [D + 1, 1024], F32, tag="pv",
                            name=f"pv_{rep}_{h}_{half}",
                        )
                    pv = pv_tiles[(h, half)]
                    for nn in (0, 512):
                        nc.tensor.matmul(
                            pv[:, nn : nn + 512],
                            v_sb[:, t, h, :],
                            pt[:, nn : nn + 512],
                            start=(t == 0),
                            stop=(t == ntc - 1),
                        )
                    if t == ntc - 1:
                        ov = opool.tile([D + 1, 1024], F32, tag="ov")
                        for nn in (0, 512):
                            nc.vector.tensor_copy(
                                ov[:, nn : nn + 512], pv[:, nn : nn + 512]
                            )
                            nc.sync.dma_start(
                                out=out_d[h, :, q0 + nn : q0 + nn + 512],
                                in_=ov[:, nn : nn + 512],
                            )

                # Interleaved emission as v1: a few v tiles + k chain A chunk 0
                # up front, everything else rides the step stream by deadline.
                PRE_V = 4
                for t in range(PRE_V):
                    proj_v_tile(t)
                proj_k_chain(0, *kchunks[0])

                work = [(t, lambda t=t: proj_v_tile(t))
                        for t in range(PRE_V, ntc)]
                for ci, (c0, c1) in enumerate(kchunks):
                    if ci == 0:
                        continue
                    # chain A chunk ci needed by (h0, half0, t=c0//128)
                    work.append(
                        (c0 // 128, lambda c0=c0, c1=c1: proj_k_chain(0, c0, c1))
                    )
                for ci, (c0, c1) in enumerate(kchunks):
                    # chain B (h2) needed from step 2*2*ntc + c0//128
                    work.append(
                        (2 * 2 * ntc + c0 // 128,
                         lambda c0=c0, c1=c1: proj_k_chain(1, c0, c1))
                    )
                work.sort(key=lambda x: x[0])

                LOOKAHEAD = lookahead
                prev = {}
                for i in range(n_steps + LOOKAHEAD):
                    if i < n_steps:
                        while work and work[0][0] <= i:
                            work.pop(0)[1]()
                        prev[i] = emit_scores(i)
                    j = i - LOOKAHEAD
                    if j >= 0:
                        pt = emit_exp(j, prev.pop(j))
                        emit_pv(j, pt)
                        if work and (j % 2 == 1):
                            work.pop(0)[1]()
                while work:
                    work.pop(0)[1]()

    nc.finalize()
    return nc


ABLATE = ()

_NC_CACHE = {}


def _get_nc(ntc, los):
    key = (ntc, tuple(los), ABLATE)
    if key not in _NC_CACHE:
        _NC_CACHE[key] = _build_nc(ntc, los, ablate=ABLATE)
    return _NC_CACHE[key]


def _host_prep(hidden_states, embx, expanded_embx, Wkv_w, Wkv_b,
               attention_mask, mlm_mask):
    hs = np.asarray(hidden_states, np.float32)
    ex = np.asarray(embx, np.float32)
    qx = np.asarray(expanded_embx, np.float32)
    w = np.asarray(Wkv_w, np.float32)
    bb = np.asarray(Wkv_b, np.float32)
    am = np.asarray(attention_mask).astype(bool)
    mm = np.asarray(mlm_mask).astype(bool)

    valid = am & ~mm                                   # (B, S)
    nvalid = valid.sum(1) + 1                          # embx key
    ntc = int(np.ceil(nvalid.max() / 128))
    skc = ntc * 128

    x = np.concatenate([ex, hs], axis=1)               # (B, S+1, HID)
    xTc = np.zeros((B, HID, skc), np.float32)
    kbf = np.full((B, skc), NEG, np.float32)
    cidx = []                                          # per-batch c(q) map
    for b in range(B):
        keep = np.concatenate([[0], 1 + np.flatnonzero(valid[b])])
        n = len(keep)
        xTc[b, :, :n] = x[b, keep].T
        kbf[b, :n] = 0.0
        c = np.zeros(S, np.int64)
        c[valid[b]] = 1 + np.arange(n - 1)             # slot of key q
        cidx.append((c, valid[b]))

    # diag windows: batch-uniform starts, data-driven values
    los = []
    dm = np.ones((B, 128, ntc, DW), np.float32)
    for t in range(ntc):
        qmins, qmaxs = [], []
        for b in range(B):
            c, vb = cidx[b]
            qs = np.flatnonzero(vb & (c // 128 == t))
            if len(qs):
                qmins.append(qs.min()); qmaxs.append(qs.max())
        lo = 0 if not qmins else min(qmins)
        lo = min(lo, S - DW)
        hi = lo if not qmaxs else max(qmaxs)
        assert hi - lo < DW, f"diag window overflow: tile {t}: {hi - lo}"
        los.append(int(lo))
        for b in range(B):
            c, vb = cidx[b]
            qs = np.flatnonzero(vb & (c // 128 == t))
            dm[b, c[qs] % 128, t, qs - lo] = 0.0

    in_maps = []
    for core in range(NCORE):
        b, g = divmod(core, 4)
        k_cols = slice(192 * g, 192 * g + 192)
        v_cols = slice(768 + 192 * g, 768 + 192 * g + 192)
        wh = [w[:, 192 * g + 64 * h : 192 * g + 64 * h + 64] for h in range(G)]
        wg = np.concatenate([wh[0], wh[1], wh[2], wh[2], w[:, v_cols]], axis=1)
        bk1 = bb[k_cols].reshape(G, D).T               # (64, 3)
        bk = np.concatenate([bk1, bk1], axis=0)        # (128, 3)
        bv = np.broadcast_to(bb[v_cols], (128, G * D))
        qtg = qx[b][:, k_cols].T                       # (192, 2048)
        kbt = kbf[b].reshape(ntc, 128).T               # (128, ntc)
        in_maps.append(dict(
            xT=np.ascontiguousarray(xTc[b].astype(ml_dtypes.bfloat16)),
            qT=np.ascontiguousarray(qtg.astype(ml_dtypes.bfloat16)),
            W=np.ascontiguousarray(wg.astype(ml_dtypes.bfloat16)),
            bk=np.ascontiguousarray(bk),
            bv=np.ascontiguousarray(bv),
            kb=np.ascontiguousarray(kbt),
            dm=np.ascontiguousarray(
                dm[b].transpose(0, 1, 2).reshape(128, ntc * DW)
                .astype(ml_dtypes.bfloat16)
            ),
        ))
    return in_maps, ntc, los


def _host_post(results):
    out = np.empty((B, S, HID), np.float32)
    for c in range(NCORE):
        b, g = divmod(c, 4)
        ot = results[c]["outT"]                        # (3, 65, 2048)
        o = ot[:, :D, :] / ot[:, D : D + 1, :]         # (3, 64, 2048)
        out[b, :, 192 * g : 192 * g + 192] = (
            o.transpose(2, 0, 1).reshape(S, G * D)
        )
    return out


def kernel(hidden_states, embx, expanded_embx, Wkv_w, Wkv_b,
           attention_mask, mlm_mask):
    in_maps, ntc, los = _host_prep(
        hidden_states, embx, expanded_embx, Wkv_w, Wkv_b,
        attention_mask, mlm_mask)
    nc = _get_nc(ntc, los)
    res = run_bass_kernel_spmd(nc, in_maps, list(range(NCORE)))
    return _host_post(res.results)
